# revision 11
# baseline (speedup 1.0000x reference)
"""LocationAwareAttention Trainium2 kernel (8-core SPMD, data-parallel over batch).

Per core: 4 batches of {S=2048, D=512}. Pipeline (per batch, transposed layout):
  hiddenT[d,s] = tanh( (W_v.T @ V.T)[d,s] + loc_conv[d,s] + (q @ W_q + bias + conv_b)[d] )
  scores[s]    = score_w . hiddenT[:,s]       (PE matmul, contraction over d)
  align        = softmax(scores)              (score_b is softmax-invariant, dropped)
  context[d]   = sum_s V.T[d,s] * align[s]    (DVE multiply-reduce)

The big matmul runs in bf16 (fp32 PSUM accumulation); fp32 elsewhere.
V is shipped pre-transposed per-core as [4, 512, 2048] f32 and cast to bf16
during the SWDGE DMA load.
"""

import numpy as np

import concourse.bass as bass
import concourse.tile as tile
from concourse import bacc, mybir
from concourse.bass_utils import run_bass_kernel_spmd

F32 = mybir.dt.float32
BF16 = mybir.dt.bfloat16
AF = mybir.ActivationFunctionType
ALU = mybir.AluOpType

NB = 4        # batches per core
S = 2048
D = 512
KC = 4        # contraction chunks of 128
MC = 4        # d-out chunks of 128
SC = 4        # s chunks of 512
SCW = 512     # s chunk width


def build_program():
    nc = bacc.Bacc("TRN2", target_bir_lowering=False, debug=False)

    vt_d = nc.dram_tensor("vt", [NB, D, S], F32, kind="ExternalInput").ap()
    qt_d = nc.dram_tensor("qt", [D, NB], F32, kind="ExternalInput").ap()
    en_d = nc.dram_tensor("energy", [NB, S], F32, kind="ExternalInput").ap()
    wq_d = nc.dram_tensor("wq", [D, D], F32, kind="ExternalInput").ap()
    wv_d = nc.dram_tensor("wv", [D, D], F32, kind="ExternalInput").ap()
    wk3_d = nc.dram_tensor("wk3", [3, D], F32, kind="ExternalInput").ap()
    bias_d = nc.dram_tensor("bias", [D], F32, kind="ExternalInput").ap()
    convb_d = nc.dram_tensor("convb", [D], F32, kind="ExternalInput").ap()
    sw_d = nc.dram_tensor("sw", [D], F32, kind="ExternalInput").ap()
    ctx_d = nc.dram_tensor("ctx_out", [NB, D], F32, kind="ExternalOutput").ap()
    align_d = nc.dram_tensor("align_out", [NB, S], F32, kind="ExternalOutput").ap()

    with tile.TileContext(nc) as tc:
        with (
            tc.tile_pool(name="const", bufs=1) as cpool,
            tc.tile_pool(name="vt", bufs=1) as vtpool,
            tc.tile_pool(name="work", bufs=2) as wpool,
            tc.tile_pool(name="hid", bufs=2) as hpool,

            tc.tile_pool(name="ps_main", bufs=3, space="PSUM") as ps_main,
            tc.tile_pool(name="ps_small", bufs=1, space="PSUM") as ps_small,
        ):
            # ---- constants (plain f32 DMA loads; casts on DVE, tiny) ----
            wv_sb = cpool.tile([128, KC * D], BF16, tag="wv")      # col block k: wv[k*128:+128, :]
            wq_sb = cpool.tile([128, KC * D], BF16, tag="wq")
            for k in range(KC):
                for src, dst in ((wv_d, wv_sb), (wq_d, wq_sb)):
                    ws = wpool.tile([128, D], F32, tag="wstage")
                    nc.sync.dma_start(ws[:, :], src[k * 128:(k + 1) * 128, :])
                    nc.vector.tensor_copy(dst[:, k * D:(k + 1) * D], ws[:, :])
            wk3f = cpool.tile([3, D], F32, tag="wk3f")
            nc.sync.dma_start(wk3f[:, :], wk3_d[:, :])
            wk3_sb = cpool.tile([3, D], BF16, tag="wk3")
            nc.vector.tensor_copy(wk3_sb[:, :], wk3f[:, :])
            swf = cpool.tile([128, MC], F32, tag="swf")
            nc.sync.dma_start(swf[:, :], sw_d.rearrange("(c p) -> p c", p=128))
            sw_sb = cpool.tile([128, MC], BF16, tag="sw")
            nc.vector.tensor_copy(sw_sb[:, :], swf[:, :])
            qtf = cpool.tile([128, KC * NB], F32, tag="qtf")
            for k in range(KC):
                nc.sync.dma_start(qtf[:, k * NB:(k + 1) * NB], qt_d[k * 128:(k + 1) * 128, :])
            qt_sb = cpool.tile([128, KC * NB], BF16, tag="qt")     # col k*NB+b
            nc.vector.tensor_copy(qt_sb[:, :], qtf[:, :])
            bias_sb = cpool.tile([128, MC], F32, tag="biasv")
            nc.sync.dma_start(bias_sb[:, :], bias_d.rearrange("(c p) -> p c", p=128))
            convb_sb = cpool.tile([128, MC], F32, tag="convb")
            nc.sync.dma_start(convb_sb[:, :], convb_d.rearrange("(c p) -> p c", p=128))
            comb_sb = cpool.tile([128, MC], F32, tag="comb")       # bias + conv_b per d
            nc.vector.tensor_add(comb_sb[:, :], bias_sb[:, :], convb_sb[:, :])

            # energy rows, all batches: load f32 once, cast to bf16 on partitions 0..3
            en_sb = cpool.tile([NB, S], F32, tag="en_f")
            nc.sync.dma_start(en_sb[:, :], en_d[:, :])
            en_bf = cpool.tile([NB, S], BF16, tag="en_bf")
            nc.vector.tensor_copy(en_bf[:, :], en_sb[:, :])

            # ---- V.T loads: f32 HWDGE DMA into staging, cast on GpSimd ----
            vt_sb = {}
            for b in range(NB):
                for k in range(KC):
                    stg = wpool.tile([128, S], F32, tag="vstage")
                    nc.sync.dma_start(stg[:, :], vt_d[b, k * 128:(k + 1) * 128, :])
                    t = vtpool.tile([128, S], BF16, tag=f"vt{b}_{k}")
                    nc.gpsimd.tensor_copy(t[:, :], stg[:, :])
                    vt_sb[(b, k)] = t

            scores_sb = cpool.tile([NB, S], F32, tag="scores")

            for b in range(NB):
                # conv energy rows: e3[k, s] = e_pad[s - 1 + k] (SBUF->SBUF DMAs)
                e3 = wpool.tile([3, S], BF16, tag="e3")
                nc.vector.memset(e3[0:3, 0:1], 0.0)
                nc.vector.memset(e3[0:3, S - 1:S], 0.0)
                nc.sync.dma_start(e3[1:2, :], en_bf[b:b + 1, :])
                nc.sync.dma_start(e3[0:1, 1:S], en_bf[b:b + 1, 0:S - 1])
                nc.sync.dma_start(e3[2:3, 0:S - 1], en_bf[b:b + 1, 1:S])

                # q-projection: qp1[1, D] = q_b @ W_q   (accumulate over k chunks)
                qp_ps = ps_small.tile([1, D], F32, tag="qp")
                for k in range(KC):
                    nc.tensor.matmul(
                        qp_ps[:, :],
                        qt_sb[:, k * NB + b:k * NB + b + 1],
                        wq_sb[:, k * D:(k + 1) * D],
                        start=(k == 0), stop=(k == KC - 1),
                    )
                qp1 = wpool.tile([1, D], F32, tag="qp1")
                nc.vector.tensor_copy(qp1[:, :], qp_ps[:, :])
                # scatter [1, 512] -> [128, 4] (d-chunk columns), then + bias + conv_b
                qpt = wpool.tile([128, MC], F32, tag="qpt")
                for m in range(MC):
                    nc.sync.dma_start(qpt[:, m:m + 1], qp1[0:1, m * 128:(m + 1) * 128])
                qb = wpool.tile([128, MC], F32, tag=f"qb{b}")
                nc.vector.tensor_add(qb[:, :], qpt[:, :], comb_sb[:, :])

                sstage = wpool.tile([1, S], F32, tag="sstage")
                for s in range(SC):
                    hid = []
                    for m in range(MC):
                        ps = ps_main.tile([128, SCW], F32, tag="main")
                        for k in range(KC):
                            nc.tensor.matmul(
                                ps[:, :],
                                wv_sb[:, k * D + m * 128:k * D + (m + 1) * 128],
                                vt_sb[(b, k)][:, s * SCW:(s + 1) * SCW],
                                start=(k == 0), stop=False,
                            )
                        nc.tensor.matmul(
                            ps[:, :],
                            wk3_sb[:, m * 128:(m + 1) * 128],
                            e3[:, s * SCW:(s + 1) * SCW],
                            start=False, stop=True,
                        )
                        h = hpool.tile([128, SCW], BF16, tag=f"hid{m}")
                        nc.scalar.activation(
                            h[:, :], ps[:, :], AF.Tanh,
                            bias=qb[:, m:m + 1], scale=1.0,
                        )
                        hid.append(h)
                    sc_ps = ps_small.tile([1, SCW], F32, tag="score")
                    for m in range(MC):
                        nc.tensor.matmul(
                            sc_ps[:, :],
                            sw_sb[:, m:m + 1],
                            hid[m][:, :],
                            start=(m == 0), stop=(m == MC - 1),
                        )
                    nc.vector.tensor_copy(sstage[0:1, s * SCW:(s + 1) * SCW], sc_ps[:, :])
                # row b of the stacked scores tile (cross-partition move => DMA)
                nc.sync.dma_start(scores_sb[b:b + 1, :], sstage[0:1, :])

            # ---- softmax over s, all batches together on partitions 0..3 ----
            mx = cpool.tile([NB, 1], F32, tag="mx")
            nc.vector.reduce_max(mx[:, :], scores_sb[:, :], axis=mybir.AxisListType.X, negate=True)
            align_f = cpool.tile([NB, S], F32, tag="align_f")
            sumex = cpool.tile([NB, 1], F32, tag="sumex")
            nc.scalar.activation(
                align_f[:, :], scores_sb[:, :], AF.Exp,
                bias=mx[:, 0:1], scale=1.0, accum_out=sumex[:, 0:1],
            )
            rcp = cpool.tile([NB, 1], F32, tag="rcp")
            nc.vector.reciprocal(rcp[:, :], sumex[:, :])
            align_n = cpool.tile([NB, S], F32, tag="align_n")
            nc.vector.tensor_scalar_mul(align_n[:, :], align_f[:, :], rcp[:, 0:1])
            nc.sync.dma_start(align_d[:, :], align_n[:, :])
            align_bf = cpool.tile([NB, S], BF16, tag="align_bf")
            nc.vector.tensor_copy(align_bf[:, :], align_n[:, :])
            # de-stack align rows into per-s-tile columns: alignT[:, t*NB+b]
            ST = S // 128  # 16 s-tiles of 128
            alignT = cpool.tile([128, ST * NB], BF16, tag="alignT")
            for b in range(NB):
                for t in range(ST):
                    nc.sync.dma_start(
                        alignT[:, t * NB + b:t * NB + b + 1],
                        align_bf[b:b + 1, t * 128:(t + 1) * 128],
                    )

            # ---- context: ctx[b, :] = align[b, :] @ V[b]  (contraction over s
            # on partitions). V straight tiles are recovered from the bf16 V.T
            # tiles with SBUF->SBUF DMA-transposes (xbar, 2-byte dtype).
            for b in range(NB):
                ctx_ps = ps_small.tile([1, D], F32, tag="ctxp")
                for t in range(ST):
                    vstr = wpool.tile([128, D], BF16, tag="vstr")
                    for c in range(KC):
                        nc.sync.dma_start(
                            vstr[:, c * 128:(c + 1) * 128],
                            vt_sb[(b, c)][:, t * 128:(t + 1) * 128],
                            transpose=True,
                        )
                    nc.tensor.matmul(
                        ctx_ps[:, :],
                        alignT[:, t * NB + b:t * NB + b + 1],
                        vstr[:, :],
                        start=(t == 0), stop=(t == ST - 1),
                    )
                ctx_row = wpool.tile([1, D], F32, tag="ctxrow")
                nc.vector.tensor_copy(ctx_row[:, :], ctx_ps[:, :])
                nc.sync.dma_start(ctx_d[b:b + 1, :], ctx_row[0:1, :])

    nc.compile()
    return nc


_NC_CACHE = None


def _get_nc():
    global _NC_CACHE
    if _NC_CACHE is None:
        _NC_CACHE = build_program()
    return _NC_CACHE


def make_in_maps(inputs):
    value = np.asarray(inputs["value"], np.float32)
    query = np.asarray(inputs["query"], np.float32)
    energy = np.asarray(inputs["last_alignment_energy"], np.float32)
    conv_w = np.asarray(inputs["conv_w"], np.float32)
    conv_b = np.asarray(inputs["conv_b"], np.float32)
    w_q = np.asarray(inputs["w_q"], np.float32)
    w_v = np.asarray(inputs["w_v"], np.float32)
    bias = np.asarray(inputs["bias"], np.float32)
    score_w = np.asarray(inputs["score_w"], np.float32)
    # score_b shifts every score equally; softmax is shift-invariant -> dropped.

    wk3 = np.ascontiguousarray(conv_w[:, 0, :].T)  # [3, D]
    in_maps = []
    for i in range(8):
        lo = i * NB
        in_maps.append({
            "vt": np.ascontiguousarray(value[lo:lo + NB].transpose(0, 2, 1)),
            "qt": np.ascontiguousarray(query[lo:lo + NB, 0, :].T),
            "energy": np.ascontiguousarray(energy[lo:lo + NB]),
            "wq": w_q, "wv": w_v, "wk3": wk3,
            "bias": bias, "convb": conv_b, "sw": score_w,
        })
    return in_maps


def kernel(**inputs):
    nc = _get_nc()
    in_maps = make_in_maps(inputs)
    res = run_bass_kernel_spmd(nc, in_maps, core_ids=list(range(8)))
    context = np.concatenate([res.results[i]["ctx_out"] for i in range(8)], axis=0)
    align = np.concatenate([res.results[i]["align_out"] for i in range(8)], axis=0)
    return context.astype(np.float32), align.astype(np.float32)


# revision 17
# speedup vs baseline: 1.6857x; 1.6857x over previous
"""LocationAwareAttention Trainium2 kernel (8-core SPMD, data-parallel over batch).

Per core: 4 batches of {S=2048, D=512}. Pipeline (per batch, transposed layout):
  hiddenT[d,s] = tanh( (W_v.T@V.T)[d,s] + conv_loc[d,s] + (q@W_q + bias + conv_b)[d] )
  scores[s]    = score_w . hiddenT[:,s]     (PE matmul, contraction over d)
  align        = softmax(scores)            (score_b is softmax-invariant, dropped)
  context      = align @ V                  (PE matmul over straight-V tiles that are
                                             recovered from V.T by xbar DMA-transpose)

The big matmuls run in bf16 (fp32 PSUM accumulation); everything else fp32.
V ships pre-transposed per-core as [4, 512, 2048] f32 (lossless layout change)
and is cast to bf16 on the GpSimd engine after a plain f32 DMA load.
"""

import numpy as np

import concourse.bass as bass
import concourse.tile as tile
from concourse import bacc, mybir
from concourse.bass_utils import run_bass_kernel_spmd
from concourse.masks import make_identity

F32 = mybir.dt.float32
BF16 = mybir.dt.bfloat16
AF = mybir.ActivationFunctionType

NB = 4        # batches per core
S = 2048
D = 512
KC = 4        # contraction chunks of 128
MC = 4        # d-out chunks of 128
SC = 4        # s chunks of 512
SCW = 512     # s chunk width
ST = S // 128  # 16 s-tiles of 128


def build_program(n_reps=1):
    """Emit the kernel. n_reps > 1 repeats the per-call body inside one NEFF
    (used only for marginal-time measurement; kernel() uses n_reps=1)."""
    nc = bacc.Bacc("TRN2", target_bir_lowering=False, debug=False)

    d = {}
    d["vt"] = nc.dram_tensor("vt", [NB, D, S], F32, kind="ExternalInput").ap()
    d["qt"] = nc.dram_tensor("qt", [D, NB], F32, kind="ExternalInput").ap()
    d["en"] = nc.dram_tensor("energy", [NB, S], F32, kind="ExternalInput").ap()
    d["wq"] = nc.dram_tensor("wq", [D, D], F32, kind="ExternalInput").ap()
    d["wv"] = nc.dram_tensor("wv", [D, D], F32, kind="ExternalInput").ap()
    d["wk3"] = nc.dram_tensor("wk3", [3, D], F32, kind="ExternalInput").ap()
    d["bias"] = nc.dram_tensor("bias", [D], F32, kind="ExternalInput").ap()
    d["convb"] = nc.dram_tensor("convb", [D], F32, kind="ExternalInput").ap()
    d["sw"] = nc.dram_tensor("sw", [D], F32, kind="ExternalInput").ap()
    d["ctx"] = nc.dram_tensor("ctx_out", [NB, D], F32, kind="ExternalOutput").ap()
    d["align"] = nc.dram_tensor("align_out", [NB, S], F32, kind="ExternalOutput").ap()

    with tile.TileContext(nc) as tc:
        with (
            tc.tile_pool(name="const", bufs=1) as cpool,
            tc.tile_pool(name="vt", bufs=1) as vtpool,
            tc.tile_pool(name="work", bufs=2) as wpool,
            tc.tile_pool(name="hid", bufs=2) as hpool,
            tc.tile_pool(name="ps_main", bufs=4, space="PSUM") as ps_main,
            tc.tile_pool(name="ps_small", bufs=1, space="PSUM") as ps_small,
        ):
            pools = (cpool, vtpool, wpool, hpool, ps_main, ps_small)
            consts = emit_consts(nc, tc, pools, d)
            for _rep in range(n_reps):
                emit_iter(nc, tc, pools, d, consts)

    nc.compile()
    return nc


def emit_consts(nc, tc, pools, d):
    cpool, vtpool, wpool, hpool, ps_main, ps_small = pools

    # weights: f32 DMA load -> DVE cast to bf16
    wv_sb = cpool.tile([128, KC * D], BF16, tag="wv")      # col block k: wv[k*128:+128, :]
    wq_sb = cpool.tile([128, KC * D], BF16, tag="wq")
    for k in range(KC):
        for src, dst in ((d["wv"], wv_sb), (d["wq"], wq_sb)):
            ws = wpool.tile([128, D], F32, tag="wstage")
            nc.sync.dma_start(ws[:, :], src[k * 128:(k + 1) * 128, :])
            nc.vector.tensor_copy(dst[:, k * D:(k + 1) * D], ws[:, :])

    # conv weights replicated at row groups 0/32/64/96 for row-packed K=3 MMs
    wk3f = cpool.tile([128, D], F32, tag="wk3f")
    nc.vector.memset(wk3f[:, :], 0.0)
    for r in range(4):
        nc.sync.dma_start(wk3f[32 * r:32 * r + 3, :], d["wk3"][:, :])
    wk3_sb = cpool.tile([128, D], BF16, tag="wk3")
    nc.vector.tensor_copy(wk3_sb[:, :], wk3f[:, :])

    swf = cpool.tile([128, MC], F32, tag="swf")
    nc.sync.dma_start(swf[:, :], d["sw"].rearrange("(c p) -> p c", p=128))
    sw_sb = cpool.tile([128, MC], BF16, tag="sw")
    nc.vector.tensor_copy(sw_sb[:, :], swf[:, :])

    qtf = cpool.tile([128, KC * NB], F32, tag="qtf")
    for k in range(KC):
        nc.sync.dma_start(qtf[:, k * NB:(k + 1) * NB], d["qt"][k * 128:(k + 1) * 128, :])
    qt_sb = cpool.tile([128, KC * NB], BF16, tag="qt")     # col k*NB+b
    nc.vector.tensor_copy(qt_sb[:, :], qtf[:, :])

    bias_sb = cpool.tile([128, MC], F32, tag="biasv")
    nc.sync.dma_start(bias_sb[:, :], d["bias"].rearrange("(c p) -> p c", p=128))
    convb_sb = cpool.tile([128, MC], F32, tag="convb")
    nc.sync.dma_start(convb_sb[:, :], d["convb"].rearrange("(c p) -> p c", p=128))
    comb_sb = cpool.tile([128, MC], F32, tag="comb")       # bias + conv_b per d
    nc.vector.tensor_add(comb_sb[:, :], bias_sb[:, :], convb_sb[:, :])

    ident = cpool.tile([128, 128], BF16, tag="ident")
    make_identity(nc, ident[:, :])
    return wv_sb, wq_sb, wk3_sb, sw_sb, qt_sb, comb_sb, ident


def emit_iter(nc, tc, pools, d, consts):
    cpool, vtpool, wpool, hpool, ps_main, ps_small = pools
    wv_sb, wq_sb, wk3_sb, sw_sb, qt_sb, comb_sb, ident = consts

    # energy rows, all batches: load f32 once, cast to bf16 on partitions 0..3
    en_f = wpool.tile([NB, S], F32, tag="vstage")
    nc.sync.dma_start(en_f[:, :], d["en"][:, :])
    en_bf = cpool.tile([NB, S], BF16, tag="en_bf")
    nc.vector.tensor_copy(en_bf[:, :], en_f[:, :])

    # V.T loads: f32 HWDGE DMA into staging, cast to bf16 on GpSimd
    vt_sb = {}
    for b in range(NB):
        for k in range(KC):
            stg = wpool.tile([128, S], F32, tag="vstage")
            nc.sync.dma_start(stg[:, :], d["vt"][b, k * 128:(k + 1) * 128, :])
            t = vtpool.tile([128, S], BF16, tag=f"vt{b}_{k}")
            nc.gpsimd.tensor_copy(t[:, :], stg[:, :])
            vt_sb[(b, k)] = t

    # q-projection, all batches at once: qp[m-chunk cols, b] = (q_b @ W_q) chunk
    qb16 = cpool.tile([128, MC * NB], F32, tag="qb16")     # col m*NB+b
    for m in range(MC):
        qp_ps = ps_small.tile([128, NB], F32, tag="tps")
        for k in range(KC):
            nc.tensor.matmul(
                qp_ps[:, :],
                wq_sb[:, k * D + m * 128:k * D + (m + 1) * 128],
                qt_sb[:, k * NB:(k + 1) * NB],
                start=(k == 0), stop=(k == KC - 1),
            )
        # + (bias + conv_b) chunk, broadcast over the batch columns
        nc.vector.tensor_scalar_add(
            qb16[:, m * NB:(m + 1) * NB], qp_ps[:, :], comb_sb[:, m:m + 1])

    scores_sb = cpool.tile([NB, S], F32, tag="scores")

    for b in range(NB):
        # conv energy rows e3[j, s] = e_pad[s - 1 + j], replicated at row
        # groups 0/32/64/96 (for the row-packed loc matmuls).
        e3 = wpool.tile([128, S], BF16, tag="e3")
        nc.vector.memset(e3[:, 0:1], 0.0)
        nc.vector.memset(e3[:, S - 1:S], 0.0)
        nc.scalar.dma_start(e3[1:2, :], en_bf[b:b + 1, :])
        nc.scalar.dma_start(e3[0:1, 1:S], en_bf[b:b + 1, 0:S - 1])
        nc.scalar.dma_start(e3[2:3, 0:S - 1], en_bf[b:b + 1, 1:S])
        for r in range(1, 4):
            nc.scalar.dma_start(e3[32 * r:32 * r + 3, :], e3[0:3, :])

        sstage = wpool.tile([1, S], F32, tag="sstage")
        for s in range(SC):
            # 4 d-chunk PSUM accumulation groups, then row-packed loc closers
            pss = []
            for m in range(MC):
                ps = ps_main.tile([128, SCW], F32, tag="main")
                for k in range(KC):
                    nc.tensor.matmul(
                        ps[:, :],
                        wv_sb[:, k * D + m * 128:k * D + (m + 1) * 128],
                        vt_sb[(b, k)][:, s * SCW:(s + 1) * SCW],
                        start=(k == 0), stop=False,
                    )
                pss.append(ps)
            for r in range(MC):
                nc.tensor.matmul(
                    pss[r][:, :],
                    wk3_sb[32 * r:32 * r + 3, r * 128:(r + 1) * 128],
                    e3[32 * r:32 * r + 3, s * SCW:(s + 1) * SCW],
                    start=False, stop=True,
                    tile_position=(32 * r, 0),
                )
            hid = []
            for m in range(MC):
                h = hpool.tile([128, SCW], BF16, tag=f"hid{m}")
                nc.scalar.activation(
                    h[:, :], pss[m][:, :], AF.Tanh,
                    bias=qb16[:, m * NB + b:m * NB + b + 1], scale=1.0,
                )
                hid.append(h)
            sc_ps = ps_small.tile([1, SCW], F32, tag="score")
            for m in range(MC):
                nc.tensor.matmul(
                    sc_ps[:, :],
                    sw_sb[:, m:m + 1],
                    hid[m][:, :],
                    start=(m == 0), stop=(m == MC - 1),
                )
            nc.vector.tensor_copy(sstage[0:1, s * SCW:(s + 1) * SCW], sc_ps[:, :])
        # row b of the stacked scores tile (cross-partition move => DMA)
        nc.sync.dma_start(scores_sb[b:b + 1, :], sstage[0:1, :])

    # softmax over s, all batches together on partitions 0..3
    mx = cpool.tile([NB, 1], F32, tag="mx")
    nc.vector.reduce_max(mx[:, :], scores_sb[:, :], axis=mybir.AxisListType.X, negate=True)
    align_f = cpool.tile([NB, S], F32, tag="align_f")
    sumex = cpool.tile([NB, 1], F32, tag="sumex")
    nc.scalar.activation(
        align_f[:, :], scores_sb[:, :], AF.Exp,
        bias=mx[:, 0:1], scale=1.0, accum_out=sumex[:, 0:1],
    )
    rcp = cpool.tile([NB, 1], F32, tag="rcp")
    nc.vector.reciprocal(rcp[:, :], sumex[:, :])
    align_n = wpool.tile([NB, S], F32, tag="vstage")
    nc.vector.tensor_scalar_mul(align_n[:, :], align_f[:, :], rcp[:, 0:1])
    nc.sync.dma_start(d["align"][:, :], align_n[:, :])
    align_bf = cpool.tile([NB, S], BF16, tag="align_bf")
    nc.vector.tensor_copy(align_bf[:, :], align_n[:, :])

    # alignT[:, t*NB+b] = align_bf[b, t*128:(t+1)*128] via PE transposes
    alignT = cpool.tile([128, ST * NB], BF16, tag="alignT")
    for t in range(ST):
        atp = ps_small.tile([128, NB], BF16, tag="tps")
        nc.tensor.transpose(atp[:, :], align_bf[0:NB, t * 128:(t + 1) * 128],
                            ident[0:NB, 0:NB])
        nc.vector.tensor_copy(alignT[:, t * NB:(t + 1) * NB], atp[:, :])

    # context: straight-V tiles via one xbar DMA-transpose per (b, d-chunk),
    # then ctx[b, :] = sum_t alignT_col(t, b).T @ Vstraight[t]
    for b in range(NB):
        vstr = wpool.tile([128, ST * 512], BF16, tag="vstr")
        for c in range(KC):
            out_ap = vstr[:, :].rearrange("p (t x) -> p t x", x=512)[:, :, c * 128:(c + 1) * 128]
            nc.scalar.dma_start_transpose(out_ap, vt_sb[(b, c)][:, :])
        ctx_ps = ps_small.tile([1, D], F32, tag="ctxp")
        for t in range(ST):
            nc.tensor.matmul(
                ctx_ps[:, :],
                alignT[:, t * NB + b:t * NB + b + 1],
                vstr[:, t * 512:(t + 1) * 512],
                start=(t == 0), stop=(t == ST - 1),
            )
        ctx_row = wpool.tile([1, D], F32, tag="ctxrow")
        nc.vector.tensor_copy(ctx_row[:, :], ctx_ps[:, :])
        nc.sync.dma_start(d["ctx"][b:b + 1, :], ctx_row[0:1, :])


_NC_CACHE = None


def _get_nc():
    global _NC_CACHE
    if _NC_CACHE is None:
        _NC_CACHE = build_program()
    return _NC_CACHE


def make_in_maps(inputs):
    value = np.asarray(inputs["value"], np.float32)
    query = np.asarray(inputs["query"], np.float32)
    energy = np.asarray(inputs["last_alignment_energy"], np.float32)
    conv_w = np.asarray(inputs["conv_w"], np.float32)
    conv_b = np.asarray(inputs["conv_b"], np.float32)
    w_q = np.asarray(inputs["w_q"], np.float32)
    w_v = np.asarray(inputs["w_v"], np.float32)
    bias = np.asarray(inputs["bias"], np.float32)
    score_w = np.asarray(inputs["score_w"], np.float32)
    # score_b shifts every score equally; softmax is shift-invariant -> dropped.

    wk3 = np.ascontiguousarray(conv_w[:, 0, :].T)  # [3, D]
    in_maps = []
    for i in range(8):
        lo = i * NB
        in_maps.append({
            "vt": np.ascontiguousarray(value[lo:lo + NB].transpose(0, 2, 1)),
            "qt": np.ascontiguousarray(query[lo:lo + NB, 0, :].T),
            "energy": np.ascontiguousarray(energy[lo:lo + NB]),
            "wq": w_q, "wv": w_v, "wk3": wk3,
            "bias": bias, "convb": conv_b, "sw": score_w,
        })
    return in_maps


def kernel(**inputs):
    nc = _get_nc()
    in_maps = make_in_maps(inputs)
    res = run_bass_kernel_spmd(nc, in_maps, core_ids=list(range(8)))
    context = np.concatenate([res.results[i]["ctx_out"] for i in range(8)], axis=0)
    align = np.concatenate([res.results[i]["align_out"] for i in range(8)], axis=0)
    return context.astype(np.float32), align.astype(np.float32)


# revision 18
# speedup vs baseline: 5.4564x; 3.2370x over previous
"""LocationAwareAttention Trainium2 kernel (8-core SPMD, data-parallel over batch).

Per core: 4 batches of {S=2048, D=512}. Pipeline (per batch, transposed layout):
  hiddenT[d,s] = tanh( (W_v.T@V.T)[d,s] + conv_loc[d,s] + (q@W_q + bias + conv_b)[d] )
  scores[s]    = score_w . hiddenT[:,s]     (PE matmul, contraction over d)
  align        = softmax(scores)            (score_b is softmax-invariant, dropped)
  context      = align @ V                  (PE matmul over straight-V tiles that are
                                             recovered from V.T by xbar DMA-transpose)

The big matmuls run in bf16 (fp32 PSUM accumulation); everything else fp32.
V ships pre-transposed per-core as [4, 512, 2048] f32 (lossless layout change)
and is cast to bf16 on the GpSimd engine after a plain f32 DMA load.
"""

import numpy as np

import concourse.bass as bass
import concourse.tile as tile
from concourse import bacc, mybir
from concourse.bass_utils import run_bass_kernel_spmd
from concourse.masks import make_identity

F32 = mybir.dt.float32
BF16 = mybir.dt.bfloat16
AF = mybir.ActivationFunctionType

NB = 4        # batches per core
S = 2048
D = 512
KC = 4        # contraction chunks of 128
MC = 4        # d-out chunks of 128
SC = 4        # s chunks of 512
SCW = 512     # s chunk width
ST = S // 128  # 16 s-tiles of 128


def build_program(n_reps=1, internal_io=False):
    """Emit the kernel. n_reps > 1 repeats the per-call body inside one NEFF;
    internal_io=True makes inputs Internal DRAM (garbage contents) so timing
    runs don't pay host->device transfer. kernel() uses the defaults."""
    nc = bacc.Bacc("TRN2", target_bir_lowering=False, debug=False)

    kind = "Internal" if internal_io else "ExternalInput"
    d = {}
    d["vt"] = nc.dram_tensor("vt", [NB, D, S], F32, kind=kind).ap()
    d["qt"] = nc.dram_tensor("qt", [D, NB], F32, kind=kind).ap()
    d["en"] = nc.dram_tensor("energy", [NB, S], F32, kind=kind).ap()
    d["wq"] = nc.dram_tensor("wq", [D, D], F32, kind=kind).ap()
    d["wv"] = nc.dram_tensor("wv", [D, D], F32, kind=kind).ap()
    d["wk3"] = nc.dram_tensor("wk3", [3, D], F32, kind=kind).ap()
    d["bias"] = nc.dram_tensor("bias", [D], F32, kind=kind).ap()
    d["convb"] = nc.dram_tensor("convb", [D], F32, kind=kind).ap()
    d["sw"] = nc.dram_tensor("sw", [D], F32, kind=kind).ap()
    d["ctx"] = nc.dram_tensor("ctx_out", [NB, D], F32, kind="ExternalOutput").ap()
    d["align"] = nc.dram_tensor("align_out", [NB, S], F32, kind="ExternalOutput").ap()

    with tile.TileContext(nc) as tc:
        with (
            tc.tile_pool(name="const", bufs=1) as cpool,
            tc.tile_pool(name="vt", bufs=1) as vtpool,
            tc.tile_pool(name="work", bufs=2) as wpool,
            tc.tile_pool(name="hid", bufs=2) as hpool,
            tc.tile_pool(name="ps_main", bufs=4, space="PSUM") as ps_main,
            tc.tile_pool(name="ps_small", bufs=1, space="PSUM") as ps_small,
        ):
            pools = (cpool, vtpool, wpool, hpool, ps_main, ps_small)
            consts = emit_consts(nc, tc, pools, d)
            for _rep in range(n_reps):
                emit_iter(nc, tc, pools, d, consts)

    nc.compile()
    return nc


def emit_consts(nc, tc, pools, d):
    cpool, vtpool, wpool, hpool, ps_main, ps_small = pools

    # weights: f32 DMA load -> DVE cast to bf16
    wv_sb = cpool.tile([128, KC * D], BF16, tag="wv")      # col block k: wv[k*128:+128, :]
    wq_sb = cpool.tile([128, KC * D], BF16, tag="wq")
    for k in range(KC):
        for src, dst in ((d["wv"], wv_sb), (d["wq"], wq_sb)):
            ws = wpool.tile([128, D], F32, tag="wstage")
            nc.sync.dma_start(ws[:, :], src[k * 128:(k + 1) * 128, :])
            nc.vector.tensor_copy(dst[:, k * D:(k + 1) * D], ws[:, :])

    # conv weights replicated at row groups 0/32/64/96 for row-packed K=3 MMs
    wk3f = cpool.tile([128, D], F32, tag="wk3f")
    nc.vector.memset(wk3f[:, :], 0.0)
    for r in range(4):
        nc.sync.dma_start(wk3f[32 * r:32 * r + 3, :], d["wk3"][:, :])
    wk3_sb = cpool.tile([128, D], BF16, tag="wk3")
    nc.vector.tensor_copy(wk3_sb[:, :], wk3f[:, :])

    swf = cpool.tile([128, MC], F32, tag="swf")
    nc.sync.dma_start(swf[:, :], d["sw"].rearrange("(c p) -> p c", p=128))
    sw_sb = cpool.tile([128, MC], BF16, tag="sw")
    nc.vector.tensor_copy(sw_sb[:, :], swf[:, :])

    qtf = cpool.tile([128, KC * NB], F32, tag="qtf")
    for k in range(KC):
        nc.sync.dma_start(qtf[:, k * NB:(k + 1) * NB], d["qt"][k * 128:(k + 1) * 128, :])
    qt_sb = cpool.tile([128, KC * NB], BF16, tag="qt")     # col k*NB+b
    nc.vector.tensor_copy(qt_sb[:, :], qtf[:, :])

    bias_sb = cpool.tile([128, MC], F32, tag="biasv")
    nc.sync.dma_start(bias_sb[:, :], d["bias"].rearrange("(c p) -> p c", p=128))
    convb_sb = cpool.tile([128, MC], F32, tag="convb")
    nc.sync.dma_start(convb_sb[:, :], d["convb"].rearrange("(c p) -> p c", p=128))
    comb_sb = cpool.tile([128, MC], F32, tag="comb")       # bias + conv_b per d
    nc.vector.tensor_add(comb_sb[:, :], bias_sb[:, :], convb_sb[:, :])

    ident = cpool.tile([128, 128], BF16, tag="ident")
    make_identity(nc, ident[:, :])
    return wv_sb, wq_sb, wk3_sb, sw_sb, qt_sb, comb_sb, ident


def emit_iter(nc, tc, pools, d, consts):
    cpool, vtpool, wpool, hpool, ps_main, ps_small = pools
    wv_sb, wq_sb, wk3_sb, sw_sb, qt_sb, comb_sb, ident = consts

    # energy rows, all batches: load f32 once, cast to bf16 on partitions 0..3
    en_f = wpool.tile([NB, S], F32, tag="vstage")
    nc.sync.dma_start(en_f[:, :], d["en"][:, :])
    en_bf = cpool.tile([NB, S], BF16, tag="en_bf")
    nc.vector.tensor_copy(en_bf[:, :], en_f[:, :])

    # V.T loads: f32 HWDGE DMA into staging, cast to bf16 on GpSimd
    vt_sb = {}
    for b in range(NB):
        for k in range(KC):
            stg = wpool.tile([128, S], F32, tag="vstage")
            nc.sync.dma_start(stg[:, :], d["vt"][b, k * 128:(k + 1) * 128, :])
            t = vtpool.tile([128, S], BF16, tag=f"vt{b}_{k}")
            nc.gpsimd.tensor_copy(t[:, :], stg[:, :])
            vt_sb[(b, k)] = t

    # q-projection, all batches at once: qp[m-chunk cols, b] = (q_b @ W_q) chunk
    qb16 = cpool.tile([128, MC * NB], F32, tag="qb16")     # col m*NB+b
    for m in range(MC):
        qp_ps = ps_small.tile([128, NB], F32, tag="tps")
        for k in range(KC):
            nc.tensor.matmul(
                qp_ps[:, :],
                wq_sb[:, k * D + m * 128:k * D + (m + 1) * 128],
                qt_sb[:, k * NB:(k + 1) * NB],
                start=(k == 0), stop=(k == KC - 1),
            )
        # + (bias + conv_b) chunk, broadcast over the batch columns
        nc.vector.tensor_scalar_add(
            qb16[:, m * NB:(m + 1) * NB], qp_ps[:, :], comb_sb[:, m:m + 1])

    scores_sb = cpool.tile([NB, S], F32, tag="scores")

    for b in range(NB):
        # conv energy rows e3[j, s] = e_pad[s - 1 + j], replicated at row
        # groups 0/32/64/96 (for the row-packed loc matmuls).
        e3 = wpool.tile([128, S], BF16, tag="e3")
        nc.vector.memset(e3[:, 0:1], 0.0)
        nc.vector.memset(e3[:, S - 1:S], 0.0)
        nc.scalar.dma_start(e3[1:2, :], en_bf[b:b + 1, :])
        nc.scalar.dma_start(e3[0:1, 1:S], en_bf[b:b + 1, 0:S - 1])
        nc.scalar.dma_start(e3[2:3, 0:S - 1], en_bf[b:b + 1, 1:S])
        for r in range(1, 4):
            nc.scalar.dma_start(e3[32 * r:32 * r + 3, :], e3[0:3, :])

        sstage = wpool.tile([1, S], F32, tag="sstage")
        for s in range(SC):
            # 4 d-chunk PSUM accumulation groups, then row-packed loc closers
            pss = []
            for m in range(MC):
                ps = ps_main.tile([128, SCW], F32, tag="main")
                for k in range(KC):
                    nc.tensor.matmul(
                        ps[:, :],
                        wv_sb[:, k * D + m * 128:k * D + (m + 1) * 128],
                        vt_sb[(b, k)][:, s * SCW:(s + 1) * SCW],
                        start=(k == 0), stop=False,
                    )
                pss.append(ps)
            for r in range(MC):
                nc.tensor.matmul(
                    pss[r][:, :],
                    wk3_sb[32 * r:32 * r + 3, r * 128:(r + 1) * 128],
                    e3[32 * r:32 * r + 3, s * SCW:(s + 1) * SCW],
                    start=False, stop=True,
                    tile_position=(32 * r, 0),
                )
            hid = []
            for m in range(MC):
                h = hpool.tile([128, SCW], BF16, tag=f"hid{m}")
                nc.scalar.activation(
                    h[:, :], pss[m][:, :], AF.Tanh,
                    bias=qb16[:, m * NB + b:m * NB + b + 1], scale=1.0,
                )
                hid.append(h)
            sc_ps = ps_small.tile([1, SCW], F32, tag="score")
            for m in range(MC):
                nc.tensor.matmul(
                    sc_ps[:, :],
                    sw_sb[:, m:m + 1],
                    hid[m][:, :],
                    start=(m == 0), stop=(m == MC - 1),
                )
            nc.vector.tensor_copy(sstage[0:1, s * SCW:(s + 1) * SCW], sc_ps[:, :])
        # row b of the stacked scores tile (cross-partition move => DMA)
        nc.sync.dma_start(scores_sb[b:b + 1, :], sstage[0:1, :])

    # softmax over s, all batches together on partitions 0..3
    mx = cpool.tile([NB, 1], F32, tag="mx")
    nc.vector.reduce_max(mx[:, :], scores_sb[:, :], axis=mybir.AxisListType.X, negate=True)
    align_f = cpool.tile([NB, S], F32, tag="align_f")
    sumex = cpool.tile([NB, 1], F32, tag="sumex")
    nc.scalar.activation(
        align_f[:, :], scores_sb[:, :], AF.Exp,
        bias=mx[:, 0:1], scale=1.0, accum_out=sumex[:, 0:1],
    )
    rcp = cpool.tile([NB, 1], F32, tag="rcp")
    nc.vector.reciprocal(rcp[:, :], sumex[:, :])
    align_n = wpool.tile([NB, S], F32, tag="vstage")
    nc.vector.tensor_scalar_mul(align_n[:, :], align_f[:, :], rcp[:, 0:1])
    nc.sync.dma_start(d["align"][:, :], align_n[:, :])
    align_bf = cpool.tile([NB, S], BF16, tag="align_bf")
    nc.vector.tensor_copy(align_bf[:, :], align_n[:, :])

    # alignT[:, t*NB+b] = align_bf[b, t*128:(t+1)*128] via PE transposes
    alignT = cpool.tile([128, ST * NB], BF16, tag="alignT")
    for t in range(ST):
        atp = ps_small.tile([128, NB], BF16, tag="tps")
        nc.tensor.transpose(atp[:, :], align_bf[0:NB, t * 128:(t + 1) * 128],
                            ident[0:NB, 0:NB])
        nc.vector.tensor_copy(alignT[:, t * NB:(t + 1) * NB], atp[:, :])

    # context: straight-V tiles via one xbar DMA-transpose per (b, d-chunk),
    # then ctx[b, :] = sum_t alignT_col(t, b).T @ Vstraight[t]
    for b in range(NB):
        vstr = wpool.tile([128, ST * 512], BF16, tag="vstr")
        for c in range(KC):
            out_ap = vstr[:, :].rearrange("p (t x) -> p t x", x=512)[:, :, c * 128:(c + 1) * 128]
            nc.scalar.dma_start_transpose(out_ap, vt_sb[(b, c)][:, :])
        ctx_ps = ps_small.tile([1, D], F32, tag="ctxp")
        for t in range(ST):
            nc.tensor.matmul(
                ctx_ps[:, :],
                alignT[:, t * NB + b:t * NB + b + 1],
                vstr[:, t * 512:(t + 1) * 512],
                start=(t == 0), stop=(t == ST - 1),
            )
        ctx_row = wpool.tile([1, D], F32, tag="ctxrow")
        nc.vector.tensor_copy(ctx_row[:, :], ctx_ps[:, :])
        nc.sync.dma_start(d["ctx"][b:b + 1, :], ctx_row[0:1, :])


_NC_CACHE = None


def _get_nc():
    global _NC_CACHE
    if _NC_CACHE is None:
        _NC_CACHE = build_program()
    return _NC_CACHE


def make_in_maps(inputs):
    value = np.asarray(inputs["value"], np.float32)
    query = np.asarray(inputs["query"], np.float32)
    energy = np.asarray(inputs["last_alignment_energy"], np.float32)
    conv_w = np.asarray(inputs["conv_w"], np.float32)
    conv_b = np.asarray(inputs["conv_b"], np.float32)
    w_q = np.asarray(inputs["w_q"], np.float32)
    w_v = np.asarray(inputs["w_v"], np.float32)
    bias = np.asarray(inputs["bias"], np.float32)
    score_w = np.asarray(inputs["score_w"], np.float32)
    # score_b shifts every score equally; softmax is shift-invariant -> dropped.

    wk3 = np.ascontiguousarray(conv_w[:, 0, :].T)  # [3, D]
    in_maps = []
    for i in range(8):
        lo = i * NB
        in_maps.append({
            "vt": np.ascontiguousarray(value[lo:lo + NB].transpose(0, 2, 1)),
            "qt": np.ascontiguousarray(query[lo:lo + NB, 0, :].T),
            "energy": np.ascontiguousarray(energy[lo:lo + NB]),
            "wq": w_q, "wv": w_v, "wk3": wk3,
            "bias": bias, "convb": conv_b, "sw": score_w,
        })
    return in_maps


def kernel(**inputs):
    nc = _get_nc()
    in_maps = make_in_maps(inputs)
    res = run_bass_kernel_spmd(nc, in_maps, core_ids=list(range(8)))
    context = np.concatenate([res.results[i]["ctx_out"] for i in range(8)], axis=0)
    align = np.concatenate([res.results[i]["align_out"] for i in range(8)], axis=0)
    return context.astype(np.float32), align.astype(np.float32)


# revision 25
# speedup vs baseline: 53.4297x; 9.7922x over previous
"""LocationAwareAttention Trainium2 kernel (8-core SPMD, data-parallel over batch).

Per core: 4 batches of {S=2048, D=512}. Pipeline (per batch, transposed layout):
  hiddenT[d,s] = tanh( (W_v.T@V.T)[d,s] + conv_loc[d,s] + (q@W_q + bias + conv_b)[d] )
  scores[s]    = score_w . hiddenT[:,s]     (PE matmul, contraction over d)
  align        = softmax(scores)            (score_b is softmax-invariant, dropped)
  context      = align @ V                  (PE matmul over straight-V tiles that are
                                             recovered from V.T by xbar DMA-transpose)

The big matmuls run in bf16 (fp32 PSUM accumulation); everything else fp32.
V ships pre-transposed per-core as [4, 512, 2048] f32 (lossless layout change)
and is cast to bf16 on the GpSimd engine after a plain f32 DMA load.
"""

import numpy as np

import concourse.bass as bass
import concourse.tile as tile
from concourse import bacc, mybir
from concourse.bass_utils import run_bass_kernel_spmd
from concourse.masks import make_identity

F32 = mybir.dt.float32
BF16 = mybir.dt.bfloat16
AF = mybir.ActivationFunctionType

NB = 4        # batches per core
S = 2048
D = 512
KC = 4        # contraction chunks of 128
MC = 4        # d-out chunks of 128
SC = 4        # s chunks of 512
SCW = 512     # s chunk width
ST = S // 128  # 16 s-tiles of 128


def build_program(n_reps=1, internal_io=False, stages="full"):
    """Emit the kernel. n_reps > 1 repeats the per-call body inside one NEFF;
    internal_io=True makes inputs Internal DRAM (garbage contents) so timing
    runs don't pay host->device transfer. kernel() uses the defaults.
    stages: full | nocontext | main | loads  (ablation for timing bisects)."""
    nc = bacc.Bacc("TRN2", target_bir_lowering=False, debug=False)

    kind = "Internal" if internal_io else "ExternalInput"
    d = {}
    d["vt"] = nc.dram_tensor("vt", [NB, D, S], F32, kind=kind).ap()
    d["qt"] = nc.dram_tensor("qt", [D, NB], F32, kind=kind).ap()
    d["en"] = nc.dram_tensor("energy", [NB, S], F32, kind=kind).ap()
    d["wq"] = nc.dram_tensor("wq", [D, D], F32, kind=kind).ap()
    d["wv"] = nc.dram_tensor("wv", [D, D], F32, kind=kind).ap()
    d["wk3"] = nc.dram_tensor("wk3", [3, D], F32, kind=kind).ap()
    d["bias"] = nc.dram_tensor("bias", [D], F32, kind=kind).ap()
    d["convb"] = nc.dram_tensor("convb", [D], F32, kind=kind).ap()
    d["sw"] = nc.dram_tensor("sw", [D], F32, kind=kind).ap()
    d["ctx"] = nc.dram_tensor("ctx_out", [NB, D], F32, kind="ExternalOutput").ap()
    d["align"] = nc.dram_tensor("align_out", [NB, S], F32, kind="ExternalOutput").ap()

    with tile.TileContext(nc) as tc:
        with (
            tc.tile_pool(name="const", bufs=1) as cpool,
            tc.tile_pool(name="vt", bufs=1) as vtpool,
            tc.tile_pool(name="work", bufs=2) as wpool,
            tc.tile_pool(name="hid", bufs=2) as hpool,
            tc.tile_pool(name="ps_main", bufs=4, space="PSUM") as ps_main,
            tc.tile_pool(name="ps_small", bufs=1, space="PSUM") as ps_small,
        ):
            pools = (cpool, vtpool, wpool, hpool, ps_main, ps_small)
            consts = emit_consts(nc, tc, pools, d)
            for _rep in range(n_reps):
                emit_iter(nc, tc, pools, d, consts, stages)

    nc.compile()
    return nc


def emit_consts(nc, tc, pools, d):
    cpool, vtpool, wpool, hpool, ps_main, ps_small = pools

    # weights: f32 DMA load -> DVE cast to bf16
    wv_sb = cpool.tile([128, KC * D], BF16, tag="wv")      # col block k: wv[k*128:+128, :]
    wq_sb = cpool.tile([128, KC * D], BF16, tag="wq")
    for k in range(KC):
        for src, dst in ((d["wv"], wv_sb), (d["wq"], wq_sb)):
            ws = wpool.tile([128, D], F32, tag="wstage")
            nc.sync.dma_start(ws[:, :], src[k * 128:(k + 1) * 128, :])
            nc.vector.tensor_copy(dst[:, k * D:(k + 1) * D], ws[:, :])

    # conv weights replicated at row groups 0/32/64/96 for row-packed K=3 MMs
    wk3f = cpool.tile([128, D], F32, tag="wk3f")
    nc.vector.memset(wk3f[:, :], 0.0)
    for r in range(4):
        nc.sync.dma_start(wk3f[32 * r:32 * r + 3, :], d["wk3"][:, :])
    wk3_sb = cpool.tile([128, D], BF16, tag="wk3")
    nc.vector.tensor_copy(wk3_sb[:, :], wk3f[:, :])

    swf = cpool.tile([128, MC], F32, tag="swf")
    nc.sync.dma_start(swf[:, :], d["sw"].rearrange("(c p) -> p c", p=128))
    sw_sb = cpool.tile([128, MC], BF16, tag="sw")
    nc.vector.tensor_copy(sw_sb[:, :], swf[:, :])

    qtf = cpool.tile([128, KC * NB], F32, tag="qtf")
    for k in range(KC):
        nc.sync.dma_start(qtf[:, k * NB:(k + 1) * NB], d["qt"][k * 128:(k + 1) * 128, :])
    qt_sb = cpool.tile([128, KC * NB], BF16, tag="qt")     # col k*NB+b
    nc.vector.tensor_copy(qt_sb[:, :], qtf[:, :])

    bias_sb = cpool.tile([128, MC], F32, tag="biasv")
    nc.sync.dma_start(bias_sb[:, :], d["bias"].rearrange("(c p) -> p c", p=128))
    convb_sb = cpool.tile([128, MC], F32, tag="convb")
    nc.sync.dma_start(convb_sb[:, :], d["convb"].rearrange("(c p) -> p c", p=128))
    comb_sb = cpool.tile([128, MC], F32, tag="comb")       # bias + conv_b per d
    nc.vector.tensor_add(comb_sb[:, :], bias_sb[:, :], convb_sb[:, :])

    ident = cpool.tile([128, 128], BF16, tag="ident")
    make_identity(nc, ident[:, :])
    return wv_sb, wq_sb, wk3_sb, sw_sb, qt_sb, comb_sb, ident


def emit_iter(nc, tc, pools, d, consts, stages="full"):
    cpool, vtpool, wpool, hpool, ps_main, ps_small = pools
    wv_sb, wq_sb, wk3_sb, sw_sb, qt_sb, comb_sb, ident = consts

    # energy rows, all batches: load f32 once, cast to bf16 on partitions 0..3
    en_f = wpool.tile([NB, S], F32, tag="vstage")
    nc.sync.dma_start(en_f[:, :], d["en"][:, :])
    en_bf = cpool.tile([NB, S], BF16, tag="en_bf")
    nc.vector.tensor_copy(en_bf[:, :], en_f[:, :])

    # V.T loads: f32 HWDGE DMA into staging, cast to bf16 on GpSimd
    vt_sb = {}
    for b in range(NB):
        for k in range(KC):
            stg = wpool.tile([128, S], F32, tag="vstage")
            nc.sync.dma_start(stg[:, :], d["vt"][b, k * 128:(k + 1) * 128, :])
            t = vtpool.tile([128, S], BF16, tag=f"vt{b}_{k}")
            nc.gpsimd.tensor_copy(t[:, :], stg[:, :])
            vt_sb[(b, k)] = t

    if stages == "loads":
        # keep all loads + casts live: ship one slice of each vt tile to dram
        junk = wpool.tile([1, S], F32, tag="sstage")
        for b in range(NB):
            for k in range(KC):
                i = b * KC + k
                nc.vector.tensor_copy(junk[0:1, i * 128:(i + 1) * 128],
                                      vt_sb[(b, k)][0:1, 0:128])
        nc.sync.dma_start(d["align"][0:1, :], junk[0:1, :])
        nc.sync.dma_start(d["ctx"][:, :], en_f[:, 0:D])
        return

    # q-projection, all batches at once: qp[m-chunk cols, b] = (q_b @ W_q) chunk
    qb16 = cpool.tile([128, MC * NB], F32, tag="qb16")     # col m*NB+b
    for m in range(MC):
        qp_ps = ps_small.tile([128, NB], F32, tag="tps")
        for k in range(KC):
            nc.tensor.matmul(
                qp_ps[:, :],
                wq_sb[:, k * D + m * 128:k * D + (m + 1) * 128],
                qt_sb[:, k * NB:(k + 1) * NB],
                start=(k == 0), stop=(k == KC - 1),
            )
        # + (bias + conv_b) chunk, broadcast over the batch columns
        nc.vector.tensor_scalar_add(
            qb16[:, m * NB:(m + 1) * NB], qp_ps[:, :], comb_sb[:, m:m + 1])

    scores_sb = cpool.tile([NB, S], F32, tag="scores")

    for b in range(NB):
        # conv energy rows e3[j, s] = e_pad[s - 1 + j], replicated at row
        # groups 0/32/64/96 (for the row-packed loc matmuls).
        e3 = wpool.tile([128, S], BF16, tag="e3")
        nc.vector.memset(e3[:, 0:1], 0.0)
        nc.vector.memset(e3[:, S - 1:S], 0.0)
        nc.scalar.dma_start(e3[1:2, :], en_bf[b:b + 1, :])
        nc.scalar.dma_start(e3[0:1, 1:S], en_bf[b:b + 1, 0:S - 1])
        nc.scalar.dma_start(e3[2:3, 0:S - 1], en_bf[b:b + 1, 1:S])
        for r in range(1, 4):
            nc.scalar.dma_start(e3[32 * r:32 * r + 3, :], e3[0:3, :])

        sstage = wpool.tile([1, S], F32, tag="sstage")
        for s in range(SC):
            # 4 d-chunk PSUM accumulation groups, then row-packed loc closers
            pss = []
            for m in range(MC):
                ps = ps_main.tile([128, SCW], F32, tag="main")
                for k in range(KC):
                    nc.tensor.matmul(
                        ps[:, :],
                        wv_sb[:, k * D + m * 128:k * D + (m + 1) * 128],
                        vt_sb[(b, k)][:, s * SCW:(s + 1) * SCW],
                        start=(k == 0), stop=False,
                    )
                pss.append(ps)
            for r in range(MC):
                nc.tensor.matmul(
                    pss[r][:, :],
                    wk3_sb[32 * r:32 * r + 3, r * 128:(r + 1) * 128],
                    e3[32 * r:32 * r + 3, s * SCW:(s + 1) * SCW],
                    start=False, stop=True,
                    tile_position=(32 * r, 0),
                )
            hid = []
            for m in range(MC):
                h = hpool.tile([128, SCW], BF16, tag=f"hid{m}")
                nc.scalar.activation(
                    h[:, :], pss[m][:, :], AF.Tanh,
                    bias=qb16[:, m * NB + b:m * NB + b + 1], scale=1.0,
                )
                hid.append(h)
            sc_ps = ps_small.tile([1, SCW], F32, tag="score")
            for m in range(MC):
                nc.tensor.matmul(
                    sc_ps[:, :],
                    sw_sb[:, m:m + 1],
                    hid[m][:, :],
                    start=(m == 0), stop=(m == MC - 1),
                )
            nc.vector.tensor_copy(sstage[0:1, s * SCW:(s + 1) * SCW], sc_ps[:, :])
        # row b of the stacked scores tile (cross-partition move => DMA)
        if stages == "main":
            nc.sync.dma_start(d["align"][b:b + 1, :], sstage[0:1, :])
        else:
            nc.sync.dma_start(scores_sb[b:b + 1, :], sstage[0:1, :])

    if stages == "main":
        nc.sync.dma_start(d["ctx"][:, :], en_f[:, 0:D])
        return

    # softmax over s, all batches together on partitions 0..3
    mx = cpool.tile([NB, 1], F32, tag="mx")
    nc.vector.reduce_max(mx[:, :], scores_sb[:, :], axis=mybir.AxisListType.X, negate=True)
    align_f = cpool.tile([NB, S], F32, tag="align_f")
    sumex = cpool.tile([NB, 1], F32, tag="sumex")
    nc.scalar.activation(
        align_f[:, :], scores_sb[:, :], AF.Exp,
        bias=mx[:, 0:1], scale=1.0, accum_out=sumex[:, 0:1],
    )
    rcp = cpool.tile([NB, 1], F32, tag="rcp")
    nc.vector.reciprocal(rcp[:, :], sumex[:, :])
    align_n = wpool.tile([NB, S], F32, tag="vstage")
    nc.vector.tensor_scalar_mul(align_n[:, :], align_f[:, :], rcp[:, 0:1])
    nc.sync.dma_start(d["align"][:, :], align_n[:, :])
    align_bf = cpool.tile([NB, S], BF16, tag="align_bf")
    nc.vector.tensor_copy(align_bf[:, :], align_n[:, :])

    # alignT[:, t*NB+b] = align_bf[b, t*128:(t+1)*128] via PE transposes
    alignT = cpool.tile([128, ST * NB], BF16, tag="alignT")
    for t in range(ST):
        atp = ps_small.tile([128, NB], BF16, tag="tps")
        nc.tensor.transpose(atp[:, :], align_bf[0:NB, t * 128:(t + 1) * 128],
                            ident[0:NB, 0:NB])
        nc.vector.tensor_copy(alignT[:, t * NB:(t + 1) * NB], atp[:, :])

    if stages == "nocontext":
        nc.sync.dma_start(d["ctx"][:, :], align_f[:, 0:D])
        return

    # context: straight-V tiles via one xbar DMA-transpose per (b, d-chunk),
    # then ctx[b, :] = sum_t alignT_col(t, b).T @ Vstraight[t]
    for b in range(NB):
        vstr = wpool.tile([128, ST * 512], BF16, tag="vstr")
        for c in range(KC):
            out_ap = vstr[:, :].rearrange("p (t x) -> p t x", x=512)[:, :, c * 128:(c + 1) * 128]
            nc.scalar.dma_start_transpose(out_ap, vt_sb[(b, c)][:, :])
        ctx_ps = ps_small.tile([1, D], F32, tag="ctxp")
        for t in range(ST):
            nc.tensor.matmul(
                ctx_ps[:, :],
                alignT[:, t * NB + b:t * NB + b + 1],
                vstr[:, t * 512:(t + 1) * 512],
                start=(t == 0), stop=(t == ST - 1),
            )
        ctx_row = wpool.tile([1, D], F32, tag="ctxrow")
        nc.vector.tensor_copy(ctx_row[:, :], ctx_ps[:, :])
        nc.sync.dma_start(d["ctx"][b:b + 1, :], ctx_row[0:1, :])


_NC_CACHE = None


def _get_nc():
    global _NC_CACHE
    if _NC_CACHE is None:
        _NC_CACHE = build_program()
    return _NC_CACHE


def make_in_maps(inputs):
    value = np.asarray(inputs["value"], np.float32)
    query = np.asarray(inputs["query"], np.float32)
    energy = np.asarray(inputs["last_alignment_energy"], np.float32)
    conv_w = np.asarray(inputs["conv_w"], np.float32)
    conv_b = np.asarray(inputs["conv_b"], np.float32)
    w_q = np.asarray(inputs["w_q"], np.float32)
    w_v = np.asarray(inputs["w_v"], np.float32)
    bias = np.asarray(inputs["bias"], np.float32)
    score_w = np.asarray(inputs["score_w"], np.float32)
    # score_b shifts every score equally; softmax is shift-invariant -> dropped.

    wk3 = np.ascontiguousarray(conv_w[:, 0, :].T)  # [3, D]
    in_maps = []
    for i in range(8):
        lo = i * NB
        in_maps.append({
            "vt": np.ascontiguousarray(value[lo:lo + NB].transpose(0, 2, 1)),
            "qt": np.ascontiguousarray(query[lo:lo + NB, 0, :].T),
            "energy": np.ascontiguousarray(energy[lo:lo + NB]),
            "wq": w_q, "wv": w_v, "wk3": wk3,
            "bias": bias, "convb": conv_b, "sw": score_w,
        })
    return in_maps


def kernel(**inputs):
    nc = _get_nc()
    in_maps = make_in_maps(inputs)
    res = run_bass_kernel_spmd(nc, in_maps, core_ids=list(range(8)))
    context = np.concatenate([res.results[i]["ctx_out"] for i in range(8)], axis=0)
    align = np.concatenate([res.results[i]["align_out"] for i in range(8)], axis=0)
    return context.astype(np.float32), align.astype(np.float32)


# revision 29
# speedup vs baseline: 233.1823x; 4.3643x over previous
"""LocationAwareAttention Trainium2 kernel (8-core SPMD, data-parallel over batch).

Per core: 4 batches of {S=2048, D=512}. Pipeline (per batch, transposed layout):
  hiddenT[d,s] = tanh( (W_v.T@V.T)[d,s] + conv_loc[d,s] + (q@W_q + bias + conv_b)[d] )
  scores[s]    = score_w . hiddenT[:,s]     (PE matmul, contraction over d)
  align        = softmax(scores)            (score_b is softmax-invariant, dropped)
  context      = align @ V                  (PE matmul over straight-V tiles that are
                                             recovered from V.T by xbar DMA-transpose)

The big matmuls run in bf16 (fp32 PSUM accumulation); everything else fp32.
V ships pre-transposed per-core as [4, 512, 2048] f32 (lossless layout change)
and is cast to bf16 on the GpSimd engine after a plain f32 DMA load.
"""

import numpy as np

import concourse.bass as bass
import concourse.tile as tile
from concourse import bacc, mybir
from concourse.bass_utils import run_bass_kernel_spmd
from concourse.masks import make_identity

F32 = mybir.dt.float32
BF16 = mybir.dt.bfloat16
AF = mybir.ActivationFunctionType

NB = 4        # batches per core
S = 2048
D = 512
KC = 4        # contraction chunks of 128
MC = 4        # d-out chunks of 128
SC = 4        # s chunks of 512
SCW = 512     # s chunk width
ST = S // 128  # 16 s-tiles of 128


def build_program(n_reps=1, internal_io=False, stages="full"):
    """Emit the kernel. n_reps > 1 repeats the per-call body inside one NEFF;
    internal_io=True makes inputs Internal DRAM (garbage contents) so timing
    runs don't pay host->device transfer. kernel() uses the defaults.
    stages: full | nocontext | main | loads  (ablation for timing bisects)."""
    nc = bacc.Bacc("TRN2", target_bir_lowering=False, debug=False)

    kind = "Internal" if internal_io else "ExternalInput"
    d = {}
    d["vt"] = nc.dram_tensor("vt", [NB, D, S], F32, kind=kind).ap()
    d["qt"] = nc.dram_tensor("qt", [D, NB], F32, kind=kind).ap()
    d["en"] = nc.dram_tensor("energy", [NB, S], F32, kind=kind).ap()
    d["wq"] = nc.dram_tensor("wq", [D, D], F32, kind=kind).ap()
    d["wv"] = nc.dram_tensor("wv", [D, D], F32, kind=kind).ap()
    d["wk3"] = nc.dram_tensor("wk3", [3, D], F32, kind=kind).ap()
    d["bias"] = nc.dram_tensor("bias", [D], F32, kind=kind).ap()
    d["convb"] = nc.dram_tensor("convb", [D], F32, kind=kind).ap()
    d["sw"] = nc.dram_tensor("sw", [D], F32, kind=kind).ap()
    d["ctx"] = nc.dram_tensor("ctx_out", [NB, D], F32, kind="ExternalOutput").ap()
    d["align"] = nc.dram_tensor("align_out", [NB, S], F32, kind="ExternalOutput").ap()

    with tile.TileContext(nc) as tc:
        with (
            tc.tile_pool(name="const", bufs=1) as cpool,
            tc.tile_pool(name="vt", bufs=1) as vtpool,
            tc.tile_pool(name="work", bufs=2) as wpool,
            tc.tile_pool(name="hid", bufs=2) as hpool,
            tc.tile_pool(name="ps_main", bufs=4, space="PSUM") as ps_main,
            tc.tile_pool(name="ps_small", bufs=1, space="PSUM") as ps_small,
        ):
            pools = (cpool, vtpool, wpool, hpool, ps_main, ps_small)
            consts = emit_consts(nc, tc, pools, d)
            for _rep in range(n_reps):
                emit_iter(nc, tc, pools, d, consts, stages)

    nc.compile()
    return nc


def emit_consts(nc, tc, pools, d):
    cpool, vtpool, wpool, hpool, ps_main, ps_small = pools

    # weights: f32 DMA load -> DVE cast to bf16
    wv_sb = cpool.tile([128, KC * D], BF16, tag="wv")      # col block k: wv[k*128:+128, :]
    wq_sb = cpool.tile([128, KC * D], BF16, tag="wq")
    for k in range(KC):
        for src, dst in ((d["wv"], wv_sb), (d["wq"], wq_sb)):
            ws = wpool.tile([128, D], F32, tag="wstage")
            nc.sync.dma_start(ws[:, :], src[k * 128:(k + 1) * 128, :])
            nc.vector.tensor_copy(dst[:, k * D:(k + 1) * D], ws[:, :])

    # conv weights replicated at row groups 0/32/64/96 for row-packed K=3 MMs
    wk3f = cpool.tile([128, D], F32, tag="wk3f")
    nc.vector.memset(wk3f[:, :], 0.0)
    for r in range(4):
        nc.sync.dma_start(wk3f[32 * r:32 * r + 3, :], d["wk3"][:, :])
    wk3_sb = cpool.tile([128, D], BF16, tag="wk3")
    nc.vector.tensor_copy(wk3_sb[:, :], wk3f[:, :])

    swf = cpool.tile([128, MC], F32, tag="swf")
    nc.sync.dma_start(swf[:, :], d["sw"].rearrange("(c p) -> p c", p=128))
    sw_sb = cpool.tile([128, MC], BF16, tag="sw")
    nc.vector.tensor_copy(sw_sb[:, :], swf[:, :])

    qtf = cpool.tile([128, KC * NB], F32, tag="qtf")
    for k in range(KC):
        nc.sync.dma_start(qtf[:, k * NB:(k + 1) * NB], d["qt"][k * 128:(k + 1) * 128, :])
    qt_sb = cpool.tile([128, KC * NB], BF16, tag="qt")     # col k*NB+b
    nc.vector.tensor_copy(qt_sb[:, :], qtf[:, :])

    bias_sb = cpool.tile([128, MC], F32, tag="biasv")
    nc.sync.dma_start(bias_sb[:, :], d["bias"].rearrange("(c p) -> p c", p=128))
    convb_sb = cpool.tile([128, MC], F32, tag="convb")
    nc.sync.dma_start(convb_sb[:, :], d["convb"].rearrange("(c p) -> p c", p=128))
    comb_sb = cpool.tile([128, MC], F32, tag="comb")       # bias + conv_b per d
    nc.vector.tensor_add(comb_sb[:, :], bias_sb[:, :], convb_sb[:, :])

    ident = cpool.tile([128, 128], BF16, tag="ident")
    make_identity(nc, ident[:, :])
    return wv_sb, wq_sb, wk3_sb, sw_sb, qt_sb, comb_sb, ident


def emit_iter(nc, tc, pools, d, consts, stages="full"):
    cpool, vtpool, wpool, hpool, ps_main, ps_small = pools
    wv_sb, wq_sb, wk3_sb, sw_sb, qt_sb, comb_sb, ident = consts

    # energy rows, all batches: load f32 once, cast to bf16 on partitions 0..3
    en_f = wpool.tile([NB, S], F32, tag="vstage")
    nc.sync.dma_start(en_f[:, :], d["en"][:, :])
    en_bf = cpool.tile([NB, S], BF16, tag="en_bf")
    nc.vector.tensor_copy(en_bf[:, :], en_f[:, :])

    # V.T loads: f32 HWDGE DMA into staging, cast to bf16 on GpSimd.
    # Loads alternate between the SP and ACT HWDGE rings.
    vt_sb = {}
    for b in range(NB):
        for k in range(KC):
            stg = wpool.tile([128, S], F32, tag="vstage")
            nc.sync.dma_start(stg[:, :], d["vt"][b, k * 128:(k + 1) * 128, :])
            t = vtpool.tile([128, S], BF16, tag=f"vt{b}_{k}")
            nc.gpsimd.tensor_copy(t[:, :], stg[:, :])
            vt_sb[(b, k)] = t

    if stages == "loads":
        # keep all loads + casts live: ship one slice of each vt tile to dram
        junk = wpool.tile([1, S], F32, tag="sstage")
        for b in range(NB):
            for k in range(KC):
                i = b * KC + k
                nc.vector.tensor_copy(junk[0:1, i * 128:(i + 1) * 128],
                                      vt_sb[(b, k)][0:1, 0:128])
        nc.sync.dma_start(d["align"][0:1, :], junk[0:1, :])
        nc.sync.dma_start(d["ctx"][:, :], en_f[:, 0:D])
        return

    # q-projection, all batches at once: qp[m-chunk cols, b] = (q_b @ W_q) chunk
    qb16 = cpool.tile([128, MC * NB], F32, tag="qb16")     # col m*NB+b
    for m in range(MC):
        qp_ps = ps_small.tile([128, NB], F32, tag="tps")
        for k in range(KC):
            nc.tensor.matmul(
                qp_ps[:, :],
                wq_sb[:, k * D + m * 128:k * D + (m + 1) * 128],
                qt_sb[:, k * NB:(k + 1) * NB],
                start=(k == 0), stop=(k == KC - 1),
            )
        # + (bias + conv_b) chunk, broadcast over the batch columns
        nc.vector.tensor_scalar_add(
            qb16[:, m * NB:(m + 1) * NB], qp_ps[:, :], comb_sb[:, m:m + 1])

    scores_sb = cpool.tile([NB, S], F32, tag="scores")

    for b in range(NB):
        # conv energy rows e3[j, s] = e_pad[s - 1 + j], replicated at row
        # groups 0/32/64/96 (for the row-packed loc matmuls).
        e3 = wpool.tile([128, S], BF16, tag="e3")
        nc.vector.memset(e3[:, 0:1], 0.0)
        nc.vector.memset(e3[:, S - 1:S], 0.0)
        nc.scalar.dma_start(e3[1:2, :], en_bf[b:b + 1, :])
        nc.scalar.dma_start(e3[0:1, 1:S], en_bf[b:b + 1, 0:S - 1])
        nc.scalar.dma_start(e3[2:3, 0:S - 1], en_bf[b:b + 1, 1:S])
        for r in range(1, 4):
            nc.scalar.dma_start(e3[32 * r:32 * r + 3, :], e3[0:3, :])

        sstage = wpool.tile([1, S], F32, tag="sstage")
        for s in range(SC):
            # 4 d-chunk PSUM accumulation groups, then row-packed loc closers
            pss = []
            for m in range(MC):
                ps = ps_main.tile([128, SCW], F32, tag="main")
                for k in range(KC):
                    nc.tensor.matmul(
                        ps[:, :],
                        wv_sb[:, k * D + m * 128:k * D + (m + 1) * 128],
                        vt_sb[(b, k)][:, s * SCW:(s + 1) * SCW],
                        start=(k == 0), stop=False,
                    )
                pss.append(ps)
            for r in range(MC):
                nc.tensor.matmul(
                    pss[r][:, :],
                    wk3_sb[32 * r:32 * r + 3, r * 128:(r + 1) * 128],
                    e3[32 * r:32 * r + 3, s * SCW:(s + 1) * SCW],
                    start=False, stop=True,
                    tile_position=(32 * r, 0),
                )
            hid = []
            for m in range(MC):
                h = hpool.tile([128, SCW], BF16, tag=f"hid{m}")
                nc.scalar.activation(
                    h[:, :], pss[m][:, :], AF.Tanh,
                    bias=qb16[:, m * NB + b:m * NB + b + 1], scale=1.0,
                )
                hid.append(h)
            sc_ps = ps_small.tile([1, SCW], F32, tag="score")
            for m in range(MC):
                nc.tensor.matmul(
                    sc_ps[:, :],
                    sw_sb[:, m:m + 1],
                    hid[m][:, :],
                    start=(m == 0), stop=(m == MC - 1),
                )
            nc.vector.tensor_copy(sstage[0:1, s * SCW:(s + 1) * SCW], sc_ps[:, :])
        # row b of the stacked scores tile (cross-partition move => DMA)
        if stages == "main":
            nc.sync.dma_start(d["align"][b:b + 1, :], sstage[0:1, :])
        else:
            nc.sync.dma_start(scores_sb[b:b + 1, :], sstage[0:1, :])

    if stages == "main":
        nc.sync.dma_start(d["ctx"][:, :], en_f[:, 0:D])
        return

    # softmax over s, all batches together on partitions 0..3
    mx = cpool.tile([NB, 1], F32, tag="mx")
    nc.vector.reduce_max(mx[:, :], scores_sb[:, :], axis=mybir.AxisListType.X, negate=True)
    align_f = cpool.tile([NB, S], F32, tag="align_f")
    sumex = cpool.tile([NB, 1], F32, tag="sumex")
    nc.scalar.activation(
        align_f[:, :], scores_sb[:, :], AF.Exp,
        bias=mx[:, 0:1], scale=1.0, accum_out=sumex[:, 0:1],
    )
    rcp = cpool.tile([NB, 1], F32, tag="rcp")
    nc.vector.reciprocal(rcp[:, :], sumex[:, :])
    align_n = wpool.tile([NB, S], F32, tag="vstage")
    nc.vector.tensor_scalar_mul(align_n[:, :], align_f[:, :], rcp[:, 0:1])
    nc.sync.dma_start(d["align"][:, :], align_n[:, :])
    align_bf = cpool.tile([NB, S], BF16, tag="align_bf")
    nc.vector.tensor_copy(align_bf[:, :], align_n[:, :])

    # alignT[:, t*NB+b] = align_bf[b, t*128:(t+1)*128] via PE transposes
    alignT = cpool.tile([128, ST * NB], BF16, tag="alignT")
    for t in range(ST):
        atp = ps_small.tile([128, NB], BF16, tag="tps")
        nc.tensor.transpose(atp[:, :], align_bf[0:NB, t * 128:(t + 1) * 128],
                            ident[0:NB, 0:NB])
        nc.vector.tensor_copy(alignT[:, t * NB:(t + 1) * NB], atp[:, :])

    if stages == "nocontext":
        nc.sync.dma_start(d["ctx"][:, :], align_f[:, 0:D])
        return

    # context: straight-V tiles via one xbar DMA-transpose per (b, d-chunk)
    # (kept on the ACT HWDGE ring, after the load phase — concurrent
    # DMA-transpose + copy traffic corrupts data on HW), then
    # ctx[b, :] = sum_t alignT_col(t, b).T @ Vstraight[t]
    for b in range(NB):
        vstr = wpool.tile([128, ST * 512], BF16, tag="vstr")
        for c in range(KC):
            out_ap = vstr[:, :].rearrange("p (t x) -> p t x", x=512)[:, :, c * 128:(c + 1) * 128]
            nc.scalar.dma_start_transpose(out_ap, vt_sb[(b, c)][:, :])
        ctx_ps = ps_small.tile([1, D], F32, tag="ctxp")
        for t in range(ST):
            nc.tensor.matmul(
                ctx_ps[:, :],
                alignT[:, t * NB + b:t * NB + b + 1],
                vstr[:, t * 512:(t + 1) * 512],
                start=(t == 0), stop=(t == ST - 1),
            )
        ctx_row = wpool.tile([1, D], F32, tag="ctxrow")
        nc.vector.tensor_copy(ctx_row[:, :], ctx_ps[:, :])
        nc.sync.dma_start(d["ctx"][b:b + 1, :], ctx_row[0:1, :])


_NC_CACHE = None


def _get_nc():
    global _NC_CACHE
    if _NC_CACHE is None:
        _NC_CACHE = build_program()
    return _NC_CACHE


def make_in_maps(inputs):
    value = np.asarray(inputs["value"], np.float32)
    query = np.asarray(inputs["query"], np.float32)
    energy = np.asarray(inputs["last_alignment_energy"], np.float32)
    conv_w = np.asarray(inputs["conv_w"], np.float32)
    conv_b = np.asarray(inputs["conv_b"], np.float32)
    w_q = np.asarray(inputs["w_q"], np.float32)
    w_v = np.asarray(inputs["w_v"], np.float32)
    bias = np.asarray(inputs["bias"], np.float32)
    score_w = np.asarray(inputs["score_w"], np.float32)
    # score_b shifts every score equally; softmax is shift-invariant -> dropped.

    wk3 = np.ascontiguousarray(conv_w[:, 0, :].T)  # [3, D]
    in_maps = []
    for i in range(8):
        lo = i * NB
        in_maps.append({
            "vt": np.ascontiguousarray(value[lo:lo + NB].transpose(0, 2, 1)),
            "qt": np.ascontiguousarray(query[lo:lo + NB, 0, :].T),
            "energy": np.ascontiguousarray(energy[lo:lo + NB]),
            "wq": w_q, "wv": w_v, "wk3": wk3,
            "bias": bias, "convb": conv_b, "sw": score_w,
        })
    return in_maps


def kernel(**inputs):
    nc = _get_nc()
    in_maps = make_in_maps(inputs)
    res = run_bass_kernel_spmd(nc, in_maps, core_ids=list(range(8)))
    context = np.concatenate([res.results[i]["ctx_out"] for i in range(8)], axis=0)
    align = np.concatenate([res.results[i]["align_out"] for i in range(8)], axis=0)
    return context.astype(np.float32), align.astype(np.float32)
